# revision 13
# baseline (speedup 1.0000x reference)
"""Sharded GQA attention (causal + packed-segment mask) for 8 Trainium2 NeuronCores.

Strategy
--------
* Core c handles batch b = c//4 and KV heads {2*(c%4), 2*(c%4)+1} (8 query
  heads per core); the sequence dim stays unsharded.
* decoder_segment_ids are sorted, so the segment mask makes attention
  block-diagonal over contiguous segment spans.  The host splits each batch
  into runs and the device kernel does causal-only attention per segment.
  The two batches' run structures are unioned (padded) so all 8 cores run
  one SPMD program; ghost rows/cols carry zeroed K/V (and a zeroed
  denominator-ones column), so they contribute nothing -- no ghost masking
  is needed, and the only masking left is the shared tile-local causal mask
  on diagonal chunks (one [128,512] constant, applied by an
  identity-stationary matmul accumulating into the QK PSUM slab).
* All operands are bf16 (fp32 PSUM accumulation): Q/K/V/identity/causal in,
  P (exp output) and the final output out.  bf16 streams 4x faster than
  fp32 through the PE and halves DMA traffic; measured relative error vs
  the fp32 reference stays well under the 2e-2 gate.
* Per (segment, kv, t-block): S^T[s, (g,t)] tiles are built by PE matmuls
  (K-chunk stationary [d,w], Q^T moving [d, 4*w]); exp runs on ScalarE
  straight out of PSUM (no max subtraction -- logits are bounded), batched
  over two chunks per instruction to amortize ACT fixed overhead; PV uses
  P^T tiles as stationary against V chunks padded with an appended ones
  column so the softmax denominator falls out of the same matmuls; the
  final normalize is a reciprocal + broadcast tensor_tensor multiply on DVE
  fused with the PSUM->SBUF copy.
* Chunk widths are padded to multiples of 8 (not 128): partial tail chunks
  of each segment stream/exp/PV only the real columns, cutting PE and ACT
  work by the tail padding that the baseline wasted.
* Input tiles are double-buffered (bufs=2) so the next For_i iteration's
  DMA overlaps compute during hardware-loop timing.
"""

import math

import numpy as np

B, T, NQ, NKV, D = 2, 1024, 32, 8, 128
G = NQ // NKV
NCORES = 8
KV_PER_CORE = NKV // (NCORES // B)
CHUNK = 128
NEG = -1.0e9
GRPB = 2  # PSUM banks (s-chunks) per exp batch
LAG = 2   # software-pipeline depth (slabs between QK/exp and PV)
RECIP_FAST = False

_PROGRAM_CACHE = {}


def _bf16():
    import ml_dtypes
    return ml_dtypes.bfloat16


# --------------------------------------------------------------------------
# host-side structure
# --------------------------------------------------------------------------

def _runs(seg_row):
    d = np.flatnonzero(np.diff(seg_row) != 0)
    starts = np.concatenate(([0], d + 1))
    ends = np.concatenate((d + 1, [len(seg_row)]))
    return [(int(s), int(e - s)) for s, e in zip(starts, ends)]


def _structure(ids):
    runs = [_runs(np.asarray(ids[b])) for b in range(B)]
    n_seg = max(len(r) for r in runs)
    L = [max((r[i][1] for r in runs if len(r) > i), default=0) for i in range(n_seg)]
    K = [math.ceil(l / CHUNK) for l in L]
    segs = [i for i in range(n_seg) if K[i] > 0]
    # padded chunk widths (union over batches, rounded up to 8)
    W = {i: [min(CHUNK, ((min(CHUNK, L[i] - CHUNK * c) + 7) // 8) * 8)
             for c in range(K[i])] for i in segs}
    slabs = [(i, kv_i, j) for i in segs for kv_i in range(KV_PER_CORE)
             for j in range(K[i])]
    chunks = [(i, kv_i, c) for i in segs for kv_i in range(KV_PER_CORE)
              for c in range(K[i])]
    # packed column offsets
    qoff = {}
    o = 0
    for s in slabs:
        qoff[s] = o
        o += 4 * W[s[0]][s[2]]
    qcols = o
    koff = {}
    o = 0
    for c in chunks:
        koff[c] = o
        o += W[c[0]][c[2]]
    kcols = o
    VW = 136  # per-chunk vO stride (128 d + 1 ones + pad, 16B aligned)
    st = {
        "runs": runs, "L": L, "K": K, "segs": segs, "W": W,
        "slabs": slabs, "chunks": chunks,
        "qoff": qoff, "qcols": qcols, "koff": koff, "kcols": kcols,
        "VW": VW,
        "slab_idx": {s: i for i, s in enumerate(slabs)},
        "chunk_idx": {c: i for i, c in enumerate(chunks)},
    }
    return st


def _prepare_core(core, q, k, v, st):
    bf16 = _bf16()
    b = core // (NCORES // B)
    kv_heads = [KV_PER_CORE * (core % (NCORES // B)) + x for x in range(KV_PER_CORE)]
    rb = st["runs"][b]
    W, VW = st["W"], st["VW"]

    def seg_info(i):
        return rb[i] if i < len(rb) else (0, 0)

    qT = np.zeros((D, st["qcols"]), np.float32)
    for (i, kv_i, j) in st["slabs"]:
        a, lb = seg_info(i)
        w = W[i][j]
        t0 = j * CHUNK
        n_real = min(w, lb - t0)
        if n_real > 0:
            o = st["qoff"][(i, kv_i, j)]
            for g in range(G):
                h = G * kv_heads[kv_i] + g
                blk = q[b, a + t0:a + t0 + n_real, h, :]  # [n_real, D]
                qT[:, o + g * w: o + g * w + n_real] = blk.T

    kT = np.zeros((D, st["kcols"]), np.float32)
    vO = np.zeros((CHUNK, len(st["chunks"]) * VW), np.float32)
    for ci, (i, kv_i, c) in enumerate(st["chunks"]):
        a, lb = seg_info(i)
        s0 = c * CHUNK
        n_real = min(W[i][c], lb - s0)
        if n_real > 0:
            kvh = kv_heads[kv_i]
            o = st["koff"][(i, kv_i, c)]
            kT[:, o: o + n_real] = k[b, a + s0:a + s0 + n_real, kvh, :].T
            vO[:n_real, ci * VW: ci * VW + D] = v[b, a + s0:a + s0 + n_real, kvh, :]
            vO[:n_real, ci * VW + D] = 1.0

    sr = np.arange(CHUNK)
    causal = np.where(sr[:, None] > sr[None, :], np.float32(NEG), np.float32(0.0))
    cc = np.tile(causal, (1, G))  # [128, 512]

    return {"qT": qT.astype(np.float16), "kT": kT.astype(np.float16),
            "vO": vO.astype(bf16), "cc": cc.astype(bf16),
            "ident": np.eye(CHUNK, dtype=np.float32).astype(bf16)}


def _assemble(outs, st):
    full = np.zeros((B, T, NQ, D), np.float32)
    W = st["W"]
    for core in range(NCORES):
        b = core // (NCORES // B)
        kv_heads = [KV_PER_CORE * (core % (NCORES // B)) + x
                    for x in range(KV_PER_CORE)]
        res = np.asarray(outs[core], np.float32)  # [NSLAB, 128, 512]
        rb = st["runs"][b]
        for si, (i, kv_i, j) in enumerate(st["slabs"]):
            if i >= len(rb):
                continue
            a, lb = rb[i]
            w = W[i][j]
            t0 = j * CHUNK
            n_real = min(w, lb - t0)
            if n_real <= 0:
                continue
            for g in range(G):
                h = G * kv_heads[kv_i] + g
                full[b, a + t0:a + t0 + n_real, h, :] = \
                    res[si, :n_real, g * CHUNK:g * CHUNK + D]
    return full


# --------------------------------------------------------------------------
# numpy emulation of the device schedule (debug/validation only)
# --------------------------------------------------------------------------

def _numpy_schedule(ins, st):
    K, W, VW = st["K"], st["W"], st["VW"]
    qT = np.asarray(ins["qT"], np.float32)
    kT = np.asarray(ins["kT"], np.float32)
    vO = np.asarray(ins["vO"], np.float32)
    cc = np.asarray(ins["cc"], np.float32)
    out = np.zeros((len(st["slabs"]), CHUNK, 512), np.float32)
    for i in st["segs"]:
        for kv_i in range(KV_PER_CORE):
            for j in range(K[i]):
                w = W[i][j]
                si = st["slab_idx"][(i, kv_i, j)]
                ot = np.zeros((CHUNK, G, 130), np.float32)
                for c in range(j + 1):
                    ci = st["chunk_idx"][(i, kv_i, c)]
                    ko = st["koff"][(i, kv_i, c)]
                    wc = W[i][c]
                    lhsT = kT[:, ko:ko + wc]                       # [d, wc]
                    qo = st["qoff"][(i, kv_i, j)]
                    rhs = qT[:, qo:qo + 4 * w]                     # [d, 4w]
                    S = np.zeros((CHUNK, 4 * w), np.float32)
                    S[:wc] = lhsT.T @ rhs
                    if c == j:
                        m = cc.reshape(CHUNK, G, CHUNK)[:, :, :w].reshape(CHUNK, 4 * w)
                        S = S + m
                    P = np.exp(S)
                    vo = vO[:, ci * VW:ci * VW + 130]              # [s, 130]
                    for g in range(G):
                        ot[:w, g, :] += P[:, g * w:(g + 1) * w].T @ vo
                denom = ot[:, :, D:D + 1]
                with np.errstate(divide="ignore", invalid="ignore"):
                    norm = np.where(denom != 0, ot[:, :, :D] / denom, 0.0)
                for g in range(G):
                    out[si, :, g * CHUNK:g * CHUNK + D] = norm[:, g, :]
    return out


# --------------------------------------------------------------------------
# bass program
# --------------------------------------------------------------------------

def _build_program(st, loop_n=0, skip=()):
    skip = set(skip)
    import contextlib

    import concourse.bacc as bacc
    import concourse.bass as bass
    import concourse.tile as tile
    from concourse import mybir

    f32 = mybir.dt.float32
    bf16 = mybir.dt.bfloat16
    f16 = mybir.dt.float16
    K, W, VW = st["K"], st["W"], st["VW"]
    segs, slabs, chunks = st["segs"], st["slabs"], st["chunks"]

    nc = bacc.Bacc()
    qT_d = nc.dram_tensor("qT", [D, st["qcols"]], f16, kind="ExternalInput")
    kT_d = nc.dram_tensor("kT", [D, st["kcols"]], f16, kind="ExternalInput")
    vO_d = nc.dram_tensor("vO", [CHUNK, len(chunks) * VW], bf16,
                          kind="ExternalInput")
    cc_d = nc.dram_tensor("cc", [CHUNK, G * CHUNK], bf16, kind="ExternalInput")
    id_d = nc.dram_tensor("ident", [CHUNK, CHUNK], bf16, kind="ExternalInput")
    out_d = nc.dram_tensor("out", [len(slabs), CHUNK, 512], bf16,
                           kind="ExternalOutput")

    with tile.TileContext(nc) as tc:
        with tc.tile_pool(name="pin", bufs=2) as pin, \
             tc.tile_pool(name="pp", bufs=3) as pp, \
             tc.tile_pool(name="po", bufs=2) as po, \
             tc.tile_pool(name="psum_s", bufs=2, space="PSUM") as psum_s, \
             tc.tile_pool(name="psum_o", bufs=2, space="PSUM") as psum_o, \
             (tc.For_i(0, loop_n, 1) if loop_n else
              contextlib.nullcontext()):

            ident_t = pin.tile([CHUNK, CHUNK], bf16, tag="ident")
            nc.sync.dma_start(out=ident_t[:], in_=id_d[:])
            cc_t = pin.tile([CHUNK, G * CHUNK], bf16, tag="cc")
            nc.sync.dma_start(out=cc_t[:], in_=cc_d[:])

            # inputs, emitted in compute-consumption order
            kT_t = {}
            vO_t = {}
            qT_t = {}
            for i in segs:
                for kv_i in range(KV_PER_CORE):
                    kk = K[i]
                    ko = st["koff"][(i, kv_i, 0)]
                    kw = sum(W[i])
                    kt = pin.tile([D, kw], f16, tag=f"kT_{i}_{kv_i}")
                    nc.sync.dma_start(out=kt[:] if "dmain" not in skip else kt[:1, :8],
                                      in_=kT_d[:, ko:ko + kw] if "dmain" not in skip else kT_d[:1, :8])
                    kT_t[(i, kv_i)] = kt
                    ci0 = st["chunk_idx"][(i, kv_i, 0)]
                    vt = pin.tile([CHUNK, kk * VW], bf16, tag=f"vO_{i}_{kv_i}")
                    nc.sync.dma_start(out=vt[:] if "dmain" not in skip else vt[:1, :8],
                                      in_=vO_d[:, ci0 * VW:(ci0 + kk) * VW] if "dmain" not in skip else vO_d[:1, :8])
                    vO_t[(i, kv_i)] = vt
                    qo = st["qoff"][(i, kv_i, 0)]
                    qw = sum(4 * W[i][j] for j in range(kk))
                    qt = pin.tile([D, qw], f16, tag=f"qT_{i}_{kv_i}")
                    nc.sync.dma_start(out=qt[:] if "dmain" not in skip else qt[:1, :8],
                                      in_=qT_d[:, qo:qo + qw] if "dmain" not in skip else qT_d[:1, :8])
                    qT_t[(i, kv_i)] = (qt, qo)

            # ---- software-pipelined slab schedule ----
            # stage A(idx): QK (+causal mask) matmuls and batched exp for
            #   slab idx; stage B(idx): PV matmuls; stage C(idx): recip +
            #   normalize into the output staging tile (+ out-DMA on the
            #   vector queue after the last slab of each (i,kv) block).
            # B/C for slab idx-1 are emitted after A(idx) so the PE never
            # idles waiting for ACT: while ACT exps slab idx, the PE runs
            # slab idx-1's PV.
            work = [(i, kv_i, j) for i in segs
                    for kv_i in range(KV_PER_CORE) for j in range(K[i])]
            max_grp = max(-(-(j + 1) // GRPB) for (_, _, j) in work)
            pt_bufs = 3 * max_grp

            ptss = {}
            ostage_t = {}

            def stage_a(idx):
                i, kv_i, j = work[idx]
                w = W[i][j]
                kt = kT_t[(i, kv_i)]
                qt, qbase = qT_t[(i, kv_i)]
                ko0 = st["koff"][(i, kv_i, 0)]
                qsl = qt[:, st["qoff"][(i, kv_i, j)] - qbase:
                         st["qoff"][(i, kv_i, j)] - qbase + 4 * w]
                pts = []
                for c0 in range(0, j + 1, GRPB):
                    grp = list(range(c0, min(c0 + GRPB, j + 1)))
                    slab = psum_s.tile([CHUNK, GRPB, 512], f32, tag="slab", name="slab")
                    for gi, c in enumerate(grp):
                        wc = W[i][c]
                        ko = st["koff"][(i, kv_i, c)] - ko0
                        lhsT = kt[:, ko:ko + wc]
                        masked = c == j
                        if "qk" not in skip:
                            nc.tensor.matmul(
                                slab[0:wc, gi, 0:4 * w], lhsT, qsl,
                                start=True, stop=not masked)
                        else:
                            nc.tensor.matmul(
                                slab[0:8, gi, 0:8], lhsT[:, 0:8],
                                qsl[:, 0:8], start=True, stop=True)
                        if masked and "qk" not in skip:
                            ccs = cc_t[:].rearrange(
                                "p (g t) -> p g t", g=G)[:, :, 0:w]
                            nc.tensor.matmul(
                                slab[:, gi, 0:4 * w].rearrange(
                                    "p (g t) -> p g t", g=G),
                                ident_t[:], ccs,
                                start=False, stop=True)
                    pt = pp.tile([CHUNK, GRPB, 512], bf16, tag="pt",
                                 bufs=pt_bufs, name="pt")
                    if "act" not in skip:
                        nc.scalar.activation(
                            out=pt[:, 0:len(grp), 0:4 * w],
                            in_=slab[:, 0:len(grp), 0:4 * w],
                            func=mybir.ActivationFunctionType.Exp)
                    else:
                        nc.gpsimd.memset(pt[:1, 0:1, 0:8], 0.0)
                    pts.append(pt)
                ptss[idx] = pts

            def stage_bc(idx):
                i, kv_i, j = work[idx]
                w = W[i][j]
                kk = K[i]
                vt = vO_t[(i, kv_i)]
                if j == 0:
                    ostage_t[(i, kv_i)] = po.tile(
                        [CHUNK, kk, 512], bf16, tag=f"os_{i}_{kv_i}", bufs=2,
                        name="ostage")
                    if "dve" in skip:
                        nc.gpsimd.memset(ostage_t[(i, kv_i)][:1, :1, :8], 0.0)
                ostage = ostage_t[(i, kv_i)]
                ot = [psum_o.tile([CHUNK, 2, 130], f32, tag=f"ot{h}",
                                  name=f"ot{h}")
                      for h in range(2)]
                pts = ptss.pop(idx)
                for c in range(j + 1):
                    pt = pts[c // GRPB]
                    psl = pt[:, c % GRPB, :]
                    ci = st["chunk_idx"][(i, kv_i, c)] - \
                        st["chunk_idx"][(i, kv_i, 0)]
                    vsl = vt[:, ci * VW:ci * VW + 130]
                    for g in range(G):
                        # each ot bank holds two heads but forms ONE
                        # accumulation group: start clears has_written
                        # bank-wide, so only the first matmul into the
                        # bank starts and only the last one stops
                        if "pv" not in skip:
                            nc.tensor.matmul(
                                ot[g // 2][0:w, g % 2, 0:130],
                                psl[:, g * w:(g + 1) * w],
                                vsl,
                                start=(c == 0 and g % 2 == 0),
                                stop=(c == j and g % 2 == 1))
                        elif c == 0 and g % 2 == 0:
                            nc.tensor.matmul(
                                ot[g // 2][0:8, g % 2, 0:8],
                                psl[:, 0:8], vsl[:, 0:8],
                                start=True, stop=True)
                recip = po.tile([CHUNK, G], f32, tag="recip", name="recip")
                for h in range(2 if "dve" not in skip else 0):
                    rh = recip[:, 2 * h:2 * h + 2]
                    if RECIP_FAST:
                        nc.vector.reciprocal_approx_fast(out=rh,
                                                         in_=ot[h][:, :, D])
                    else:
                        nc.vector.reciprocal(out=rh, in_=ot[h][:, :, D])
                    recip_b = bass.AP(
                        tensor=rh.tensor, offset=rh.offset,
                        ap=[rh.ap[0], rh.ap[1], [0, D]])
                    nc.vector.tensor_mul(
                        out=ostage[:, j, 2 * h * 128:(2 * h + 2) * 128]
                            .rearrange("p (g d) -> p g d", g=2),
                        in0=ot[h][:, :, 0:D],
                        in1=recip_b)
                si = st["slab_idx"][(i, kv_i, j)]
                if "dmaout" not in skip:
                    nc.sync.dma_start(out=out_d[si],
                                      in_=ostage[:, j, :])
                else:
                    nc.sync.dma_start(out=out_d[si][:1, :8],
                                      in_=ostage[:1, j, :8])

            for idx in range(len(work) + LAG):
                if idx < len(work):
                    stage_a(idx)
                if idx >= LAG:
                    stage_bc(idx - LAG)

    nc.finalize()
    return nc


# --------------------------------------------------------------------------
# entry point
# --------------------------------------------------------------------------

def kernel(query, key, value, decoder_segment_ids, _trace=False, _numpy=False):
    query = np.asarray(query, np.float32)
    key = np.asarray(key, np.float32)
    value = np.asarray(value, np.float32)
    ids = np.asarray(decoder_segment_ids)
    # the block-diagonal decomposition relies on segment ids being sorted
    # (contiguous segments), as setup_inputs guarantees
    assert np.all(np.diff(ids.astype(np.int64), axis=-1) >= 0)

    st = _structure(ids)
    core_ins = [_prepare_core(c, query, key, value, st) for c in range(NCORES)]

    if _numpy:
        outs = [_numpy_schedule(ci, st) for ci in core_ins]
        return _assemble(outs, st)

    from concourse.bass_utils import run_bass_kernel_spmd

    cache_key = (tuple(st["L"]),)
    if cache_key not in _PROGRAM_CACHE:
        _PROGRAM_CACHE[cache_key] = _build_program(st)
    nc = _PROGRAM_CACHE[cache_key]

    in_maps = [dict(ci) for ci in core_ins]
    res = run_bass_kernel_spmd(nc, in_maps, list(range(NCORES)), trace=_trace)
    outs = [res.results[c]["out"] for c in range(NCORES)]
    full = _assemble(outs, st)
    if _trace:
        return full, res
    return full


# revision 14
# speedup vs baseline: 1.4005x; 1.4005x over previous
"""Sharded GQA attention (causal + packed-segment mask) for 8 Trainium2 NeuronCores.

Strategy
--------
* Core c handles batch b = c//4 and KV heads {2*(c%4), 2*(c%4)+1} (8 query
  heads per core); the sequence dim stays unsharded.
* decoder_segment_ids are sorted, so the segment mask makes attention
  block-diagonal over contiguous segment spans.  The host splits each batch
  into runs and the device kernel does causal-only attention per segment.
  The two batches' run structures are unioned (padded) so all 8 cores run
  one SPMD program; ghost rows/cols carry zeroed K/V (and a zeroed
  denominator-ones column), so they contribute nothing -- no ghost masking
  is needed, and the only masking left is the shared tile-local causal mask
  on diagonal chunks (one [128,512] constant, applied by an
  identity-stationary matmul accumulating into the QK PSUM slab).
* All operands are bf16 (fp32 PSUM accumulation): Q/K/V/identity/causal in,
  P (exp output) and the final output out.  bf16 streams 4x faster than
  fp32 through the PE and halves DMA traffic; measured relative error vs
  the fp32 reference stays well under the 2e-2 gate.
* Per (segment, kv, t-block): S^T[s, (g,t)] tiles are built by PE matmuls
  (K-chunk stationary [d,w], Q^T moving [d, 4*w]); exp runs on ScalarE
  straight out of PSUM (no max subtraction -- logits are bounded), batched
  over two chunks per instruction to amortize ACT fixed overhead; PV uses
  P^T tiles as stationary against V chunks padded with an appended ones
  column so the softmax denominator falls out of the same matmuls; the
  final normalize is a reciprocal + broadcast tensor_tensor multiply on DVE
  fused with the PSUM->SBUF copy.
* Chunk widths are padded to multiples of 8 (not 128): partial tail chunks
  of each segment stream/exp/PV only the real columns, cutting PE and ACT
  work by the tail padding that the baseline wasted.
* Input tiles are double-buffered (bufs=2) so the next For_i iteration's
  DMA overlaps compute during hardware-loop timing.
"""

import math

import numpy as np

B, T, NQ, NKV, D = 2, 1024, 32, 8, 128
G = NQ // NKV
NCORES = 8
KV_PER_CORE = NKV // (NCORES // B)
CHUNK = 128
NEG = -1.0e9
GRPB = 2  # PSUM banks (s-chunks) per exp batch
LAG = 1   # software-pipeline depth (slabs between QK/exp and PV)
RECIP_FAST = False
OUT_MODE = "block_gpsimd"  # block_gpsimd | slab_sync | slab_gpsimd

_PROGRAM_CACHE = {}


def _bf16():
    import ml_dtypes
    return ml_dtypes.bfloat16


# --------------------------------------------------------------------------
# host-side structure
# --------------------------------------------------------------------------

def _runs(seg_row):
    d = np.flatnonzero(np.diff(seg_row) != 0)
    starts = np.concatenate(([0], d + 1))
    ends = np.concatenate((d + 1, [len(seg_row)]))
    return [(int(s), int(e - s)) for s, e in zip(starts, ends)]


def _structure(ids):
    runs = [_runs(np.asarray(ids[b])) for b in range(B)]
    n_seg = max(len(r) for r in runs)
    L = [max((r[i][1] for r in runs if len(r) > i), default=0) for i in range(n_seg)]
    K = [math.ceil(l / CHUNK) for l in L]
    segs = [i for i in range(n_seg) if K[i] > 0]
    # padded chunk widths (union over batches, rounded up to 8)
    W = {i: [min(CHUNK, ((min(CHUNK, L[i] - CHUNK * c) + 7) // 8) * 8)
             for c in range(K[i])] for i in segs}
    slabs = [(i, kv_i, j) for i in segs for kv_i in range(KV_PER_CORE)
             for j in range(K[i])]
    chunks = [(i, kv_i, c) for i in segs for kv_i in range(KV_PER_CORE)
              for c in range(K[i])]
    # packed column offsets
    qoff = {}
    o = 0
    for s in slabs:
        qoff[s] = o
        o += 4 * W[s[0]][s[2]]
    qcols = o
    koff = {}
    o = 0
    for c in chunks:
        koff[c] = o
        o += W[c[0]][c[2]]
    kcols = o
    VW = 136  # per-chunk vO stride (128 d + 1 ones + pad, 16B aligned)
    st = {
        "runs": runs, "L": L, "K": K, "segs": segs, "W": W,
        "slabs": slabs, "chunks": chunks,
        "qoff": qoff, "qcols": qcols, "koff": koff, "kcols": kcols,
        "VW": VW,
        "slab_idx": {s: i for i, s in enumerate(slabs)},
        "chunk_idx": {c: i for i, c in enumerate(chunks)},
    }
    return st


def _prepare_core(core, q, k, v, st):
    bf16 = _bf16()
    b = core // (NCORES // B)
    kv_heads = [KV_PER_CORE * (core % (NCORES // B)) + x for x in range(KV_PER_CORE)]
    rb = st["runs"][b]
    W, VW = st["W"], st["VW"]

    def seg_info(i):
        return rb[i] if i < len(rb) else (0, 0)

    qT = np.zeros((D, st["qcols"]), np.float32)
    for (i, kv_i, j) in st["slabs"]:
        a, lb = seg_info(i)
        w = W[i][j]
        t0 = j * CHUNK
        n_real = min(w, lb - t0)
        if n_real > 0:
            o = st["qoff"][(i, kv_i, j)]
            for g in range(G):
                h = G * kv_heads[kv_i] + g
                blk = q[b, a + t0:a + t0 + n_real, h, :]  # [n_real, D]
                qT[:, o + g * w: o + g * w + n_real] = blk.T

    kT = np.zeros((D, st["kcols"]), np.float32)
    vO = np.zeros((CHUNK, len(st["chunks"]) * VW), np.float32)
    for ci, (i, kv_i, c) in enumerate(st["chunks"]):
        a, lb = seg_info(i)
        s0 = c * CHUNK
        n_real = min(W[i][c], lb - s0)
        if n_real > 0:
            kvh = kv_heads[kv_i]
            o = st["koff"][(i, kv_i, c)]
            kT[:, o: o + n_real] = k[b, a + s0:a + s0 + n_real, kvh, :].T
            vO[:n_real, ci * VW: ci * VW + D] = v[b, a + s0:a + s0 + n_real, kvh, :]
            vO[:n_real, ci * VW + D] = 1.0

    sr = np.arange(CHUNK)
    causal = np.where(sr[:, None] > sr[None, :], np.float32(NEG), np.float32(0.0))
    cc = np.tile(causal, (1, G))  # [128, 512]

    return {"qT": qT.astype(np.float16), "kT": kT.astype(np.float16),
            "vO": vO.astype(bf16), "cc": cc.astype(bf16),
            "ident": np.eye(CHUNK, dtype=np.float32).astype(bf16)}


def _assemble(outs, st):
    full = np.zeros((B, T, NQ, D), np.float32)
    W = st["W"]
    for core in range(NCORES):
        b = core // (NCORES // B)
        kv_heads = [KV_PER_CORE * (core % (NCORES // B)) + x
                    for x in range(KV_PER_CORE)]
        res = np.asarray(outs[core], np.float32)  # [NSLAB, 128, 512]
        rb = st["runs"][b]
        for si, (i, kv_i, j) in enumerate(st["slabs"]):
            if i >= len(rb):
                continue
            a, lb = rb[i]
            w = W[i][j]
            t0 = j * CHUNK
            n_real = min(w, lb - t0)
            if n_real <= 0:
                continue
            for g in range(G):
                h = G * kv_heads[kv_i] + g
                full[b, a + t0:a + t0 + n_real, h, :] = \
                    res[si, :n_real, g * CHUNK:g * CHUNK + D]
    return full


# --------------------------------------------------------------------------
# numpy emulation of the device schedule (debug/validation only)
# --------------------------------------------------------------------------

def _numpy_schedule(ins, st):
    K, W, VW = st["K"], st["W"], st["VW"]
    qT = np.asarray(ins["qT"], np.float32)
    kT = np.asarray(ins["kT"], np.float32)
    vO = np.asarray(ins["vO"], np.float32)
    cc = np.asarray(ins["cc"], np.float32)
    out = np.zeros((len(st["slabs"]), CHUNK, 512), np.float32)
    for i in st["segs"]:
        for kv_i in range(KV_PER_CORE):
            for j in range(K[i]):
                w = W[i][j]
                si = st["slab_idx"][(i, kv_i, j)]
                ot = np.zeros((CHUNK, G, 130), np.float32)
                for c in range(j + 1):
                    ci = st["chunk_idx"][(i, kv_i, c)]
                    ko = st["koff"][(i, kv_i, c)]
                    wc = W[i][c]
                    lhsT = kT[:, ko:ko + wc]                       # [d, wc]
                    qo = st["qoff"][(i, kv_i, j)]
                    rhs = qT[:, qo:qo + 4 * w]                     # [d, 4w]
                    S = np.zeros((CHUNK, 4 * w), np.float32)
                    S[:wc] = lhsT.T @ rhs
                    if c == j:
                        m = cc.reshape(CHUNK, G, CHUNK)[:, :, :w].reshape(CHUNK, 4 * w)
                        S = S + m
                    P = np.exp(S)
                    vo = vO[:, ci * VW:ci * VW + 130]              # [s, 130]
                    for g in range(G):
                        ot[:w, g, :] += P[:, g * w:(g + 1) * w].T @ vo
                denom = ot[:, :, D:D + 1]
                with np.errstate(divide="ignore", invalid="ignore"):
                    norm = np.where(denom != 0, ot[:, :, :D] / denom, 0.0)
                for g in range(G):
                    out[si, :, g * CHUNK:g * CHUNK + D] = norm[:, g, :]
    return out


# --------------------------------------------------------------------------
# bass program
# --------------------------------------------------------------------------

def _build_program(st, loop_n=0, skip=()):
    skip = set(skip)
    import contextlib

    import concourse.bacc as bacc
    import concourse.bass as bass
    import concourse.tile as tile
    from concourse import mybir

    f32 = mybir.dt.float32
    bf16 = mybir.dt.bfloat16
    f16 = mybir.dt.float16
    K, W, VW = st["K"], st["W"], st["VW"]
    segs, slabs, chunks = st["segs"], st["slabs"], st["chunks"]

    nc = bacc.Bacc()
    qT_d = nc.dram_tensor("qT", [D, st["qcols"]], f16, kind="ExternalInput")
    kT_d = nc.dram_tensor("kT", [D, st["kcols"]], f16, kind="ExternalInput")
    vO_d = nc.dram_tensor("vO", [CHUNK, len(chunks) * VW], bf16,
                          kind="ExternalInput")
    cc_d = nc.dram_tensor("cc", [CHUNK, G * CHUNK], bf16, kind="ExternalInput")
    id_d = nc.dram_tensor("ident", [CHUNK, CHUNK], bf16, kind="ExternalInput")
    out_d = nc.dram_tensor("out", [len(slabs), CHUNK, 512], bf16,
                           kind="ExternalOutput")

    with tile.TileContext(nc) as tc:
        with tc.tile_pool(name="pin", bufs=2) as pin, \
             tc.tile_pool(name="pp", bufs=3) as pp, \
             tc.tile_pool(name="po", bufs=2) as po, \
             tc.tile_pool(name="psum_s", bufs=2, space="PSUM") as psum_s, \
             tc.tile_pool(name="psum_o", bufs=2, space="PSUM") as psum_o, \
             (tc.For_i(0, loop_n, 1) if loop_n else
              contextlib.nullcontext()):

            ident_t = pin.tile([CHUNK, CHUNK], bf16, tag="ident")
            nc.sync.dma_start(out=ident_t[:], in_=id_d[:])
            cc_t = pin.tile([CHUNK, G * CHUNK], bf16, tag="cc")
            nc.sync.dma_start(out=cc_t[:], in_=cc_d[:])

            # inputs, emitted in compute-consumption order
            kT_t = {}
            vO_t = {}
            qT_t = {}
            for i in segs:
                for kv_i in range(KV_PER_CORE):
                    kk = K[i]
                    ko = st["koff"][(i, kv_i, 0)]
                    kw = sum(W[i])
                    kt = pin.tile([D, kw], f16, tag=f"kT_{i}_{kv_i}")
                    nc.sync.dma_start(out=kt[:] if "dmain" not in skip else kt[:1, :8],
                                      in_=kT_d[:, ko:ko + kw] if "dmain" not in skip else kT_d[:1, :8])
                    kT_t[(i, kv_i)] = kt
                    ci0 = st["chunk_idx"][(i, kv_i, 0)]
                    vt = pin.tile([CHUNK, kk * VW], bf16, tag=f"vO_{i}_{kv_i}")
                    nc.sync.dma_start(out=vt[:] if "dmain" not in skip else vt[:1, :8],
                                      in_=vO_d[:, ci0 * VW:(ci0 + kk) * VW] if "dmain" not in skip else vO_d[:1, :8])
                    vO_t[(i, kv_i)] = vt
                    qo = st["qoff"][(i, kv_i, 0)]
                    qw = sum(4 * W[i][j] for j in range(kk))
                    qt = pin.tile([D, qw], f16, tag=f"qT_{i}_{kv_i}")
                    nc.sync.dma_start(out=qt[:] if "dmain" not in skip else qt[:1, :8],
                                      in_=qT_d[:, qo:qo + qw] if "dmain" not in skip else qT_d[:1, :8])
                    qT_t[(i, kv_i)] = (qt, qo)

            # ---- software-pipelined slab schedule ----
            # stage A(idx): QK (+causal mask) matmuls and batched exp for
            #   slab idx; stage B(idx): PV matmuls; stage C(idx): recip +
            #   normalize into the output staging tile (+ out-DMA on the
            #   vector queue after the last slab of each (i,kv) block).
            # B/C for slab idx-1 are emitted after A(idx) so the PE never
            # idles waiting for ACT: while ACT exps slab idx, the PE runs
            # slab idx-1's PV.
            work = [(i, kv_i, j) for i in segs
                    for kv_i in range(KV_PER_CORE) for j in range(K[i])]
            max_grp = max(-(-(j + 1) // GRPB) for (_, _, j) in work)
            pt_bufs = (LAG + 1) * max_grp

            ptss = {}
            ostage_t = {}

            def stage_a(idx):
                i, kv_i, j = work[idx]
                w = W[i][j]
                kt = kT_t[(i, kv_i)]
                qt, qbase = qT_t[(i, kv_i)]
                ko0 = st["koff"][(i, kv_i, 0)]
                qsl = qt[:, st["qoff"][(i, kv_i, j)] - qbase:
                         st["qoff"][(i, kv_i, j)] - qbase + 4 * w]
                pts = []
                for c0 in range(0, j + 1, GRPB):
                    grp = list(range(c0, min(c0 + GRPB, j + 1)))
                    slab = psum_s.tile([CHUNK, GRPB, 512], f32, tag="slab", name="slab")
                    for gi, c in enumerate(grp):
                        wc = W[i][c]
                        ko = st["koff"][(i, kv_i, c)] - ko0
                        lhsT = kt[:, ko:ko + wc]
                        masked = c == j
                        if "qk" not in skip:
                            nc.tensor.matmul(
                                slab[0:wc, gi, 0:4 * w], lhsT, qsl,
                                start=True, stop=not masked)
                        else:
                            nc.tensor.matmul(
                                slab[0:8, gi, 0:8], lhsT[:, 0:8],
                                qsl[:, 0:8], start=True, stop=True)
                        if masked and "qk" not in skip:
                            ccs = cc_t[:].rearrange(
                                "p (g t) -> p g t", g=G)[:, :, 0:w]
                            nc.tensor.matmul(
                                slab[:, gi, 0:4 * w].rearrange(
                                    "p (g t) -> p g t", g=G),
                                ident_t[:], ccs,
                                start=False, stop=True)
                    pt = pp.tile([CHUNK, GRPB, 512], bf16, tag="pt",
                                 bufs=pt_bufs, name="pt")
                    if "act" not in skip:
                        nc.scalar.activation(
                            out=pt[:, 0:len(grp), 0:4 * w],
                            in_=slab[:, 0:len(grp), 0:4 * w],
                            func=mybir.ActivationFunctionType.Exp)
                    else:
                        nc.gpsimd.memset(pt[:1, 0:1, 0:8], 0.0)
                    pts.append(pt)
                ptss[idx] = pts

            def stage_bc(idx):
                i, kv_i, j = work[idx]
                w = W[i][j]
                kk = K[i]
                vt = vO_t[(i, kv_i)]
                if j == 0:
                    ostage_t[(i, kv_i)] = po.tile(
                        [CHUNK, kk, 512], bf16, tag=f"os_{i}_{kv_i}", bufs=2,
                        name="ostage")
                    if "dve" in skip:
                        nc.gpsimd.memset(ostage_t[(i, kv_i)][:1, :1, :8], 0.0)
                ostage = ostage_t[(i, kv_i)]
                ot = [psum_o.tile([CHUNK, 2, 130], f32, tag=f"ot{h}",
                                  name=f"ot{h}")
                      for h in range(2)]
                pts = ptss.pop(idx)
                for c in range(j + 1):
                    pt = pts[c // GRPB]
                    psl = pt[:, c % GRPB, :]
                    ci = st["chunk_idx"][(i, kv_i, c)] - \
                        st["chunk_idx"][(i, kv_i, 0)]
                    vsl = vt[:, ci * VW:ci * VW + 130]
                    for g in range(G):
                        # each ot bank holds two heads but forms ONE
                        # accumulation group: start clears has_written
                        # bank-wide, so only the first matmul into the
                        # bank starts and only the last one stops
                        if "pv" not in skip:
                            nc.tensor.matmul(
                                ot[g // 2][0:w, g % 2, 0:130],
                                psl[:, g * w:(g + 1) * w],
                                vsl,
                                start=(c == 0 and g % 2 == 0),
                                stop=(c == j and g % 2 == 1))
                        elif c == 0 and g % 2 == 0:
                            nc.tensor.matmul(
                                ot[g // 2][0:8, g % 2, 0:8],
                                psl[:, 0:8], vsl[:, 0:8],
                                start=True, stop=True)
                recip = po.tile([CHUNK, G], f32, tag="recip", name="recip")
                for h in range(2 if "dve" not in skip else 0):
                    rh = recip[:, 2 * h:2 * h + 2]
                    if RECIP_FAST:
                        nc.vector.reciprocal_approx_fast(out=rh,
                                                         in_=ot[h][:, :, D])
                    else:
                        nc.vector.reciprocal(out=rh, in_=ot[h][:, :, D])
                    recip_b = bass.AP(
                        tensor=rh.tensor, offset=rh.offset,
                        ap=[rh.ap[0], rh.ap[1], [0, D]])
                    nc.vector.tensor_mul(
                        out=ostage[:, j, 2 * h * 128:(2 * h + 2) * 128]
                            .rearrange("p (g d) -> p g d", g=2),
                        in0=ot[h][:, :, 0:D],
                        in1=recip_b)
                si = st["slab_idx"][(i, kv_i, j)]
                if OUT_MODE == "block_gpsimd":
                    if j == kk - 1:
                        si0 = st["slab_idx"][(i, kv_i, 0)]
                        if "dmaout" not in skip:
                            nc.gpsimd.dma_start(
                                out=out_d[si0:si0 + kk].rearrange(
                                    "k p c -> p k c"),
                                in_=ostage[:])
                        else:
                            nc.gpsimd.dma_start(out=out_d[si0][:1, :8],
                                                in_=ostage[:1, :1, :8])
                else:
                    eng = nc.sync if OUT_MODE == "slab_sync" else nc.gpsimd
                    if "dmaout" not in skip:
                        eng.dma_start(out=out_d[si], in_=ostage[:, j, :])
                    else:
                        eng.dma_start(out=out_d[si][:1, :8],
                                      in_=ostage[:1, j, :8])

            for idx in range(len(work) + LAG):
                if idx < len(work):
                    stage_a(idx)
                if idx >= LAG:
                    stage_bc(idx - LAG)

    nc.finalize()
    return nc


# --------------------------------------------------------------------------
# entry point
# --------------------------------------------------------------------------

def kernel(query, key, value, decoder_segment_ids, _trace=False, _numpy=False):
    query = np.asarray(query, np.float32)
    key = np.asarray(key, np.float32)
    value = np.asarray(value, np.float32)
    ids = np.asarray(decoder_segment_ids)
    # the block-diagonal decomposition relies on segment ids being sorted
    # (contiguous segments), as setup_inputs guarantees
    assert np.all(np.diff(ids.astype(np.int64), axis=-1) >= 0)

    st = _structure(ids)
    core_ins = [_prepare_core(c, query, key, value, st) for c in range(NCORES)]

    if _numpy:
        outs = [_numpy_schedule(ci, st) for ci in core_ins]
        return _assemble(outs, st)

    from concourse.bass_utils import run_bass_kernel_spmd

    cache_key = (tuple(st["L"]),)
    if cache_key not in _PROGRAM_CACHE:
        _PROGRAM_CACHE[cache_key] = _build_program(st)
    nc = _PROGRAM_CACHE[cache_key]

    in_maps = [dict(ci) for ci in core_ins]
    res = run_bass_kernel_spmd(nc, in_maps, list(range(NCORES)), trace=_trace)
    outs = [res.results[c]["out"] for c in range(NCORES)]
    full = _assemble(outs, st)
    if _trace:
        return full, res
    return full
